# revision 4
# baseline (speedup 1.0000x reference)
"""Trainium2 Bass kernel for nn_CrossLayer: out = LayerNorm(x0 * (x1@w) + x0).

Math: s = x1 @ w (per-row scalar), y = x0*s + x0, out = LN(y)*gamma + beta.
Per 128-row tile (H=2048 free dim):
    DVE : scalar_tensor_tensor + accum -> s = rowsum((x1*1)*w_b)  (1 pass, out
          is a stride-0 dummy so the product never lands in SBUF)
    DVE : scalar_tensor_tensor + accum -> y = (x0*s)+x0, ysum     (1 pass;
          bit-identical rounding to the reference's x0*s + x0)
    ACT : activation(Square, bias=-mean, accum) -> ss = sum((y-mean)^2)
    tiny: rstd = 1/sqrt(ss/H + eps);  b = -mean*rstd
    ACT : activation(Identity, scale=rstd, bias=b) -> out = y*rstd + b
Sharding: pure data parallel, rows split across 8 cores; weight/gamma/beta
replicated. gamma==1/beta==0 detected host-side and folded away (the general
affine path applies two extra vector passes).
"""

import numpy as np

B, H = 16384, 2048
N_CORES = 8
ROWS = B // N_CORES          # rows per core
P = 128                      # partitions
NT = ROWS // P               # tiles per core
LN_EPS = 1e-12

_cache = {}


def _build(apply_affine: bool):
    import concourse.bass as bass
    import concourse.bacc as bacc
    import concourse.tile as tile
    from concourse import mybir

    f32 = mybir.dt.float32
    op = mybir.AluOpType
    act_fn = mybir.ActivationFunctionType

    nc = bacc.Bacc("TRN2", target_bir_lowering=False, debug=False)
    x0 = nc.dram_tensor("x0", [ROWS, H], f32, kind="ExternalInput")
    x1 = nc.dram_tensor("x1", [ROWS, H], f32, kind="ExternalInput")
    w = nc.dram_tensor("weight", [H, 1], f32, kind="ExternalInput")
    if apply_affine:
        gamma = nc.dram_tensor("ln_gamma", [H], f32, kind="ExternalInput")
        beta = nc.dram_tensor("ln_beta", [H], f32, kind="ExternalInput")
    out = nc.dram_tensor("out", [ROWS, H], f32, kind="ExternalOutput")

    def bcast_rows(ap_1d):
        # [H] DRAM vector -> [P, H] SBUF tile via partition-stride-0 DMA
        return bass.AP(
            tensor=ap_1d.tensor,
            offset=ap_1d.offset,
            ap=[[0, P]] + list(ap_1d.ap),
        )

    with tile.TileContext(nc) as tc:
        with (
            tc.tile_pool(name="singles", bufs=1) as singles,
            tc.tile_pool(name="io", bufs=3) as io,
            tc.tile_pool(name="work", bufs=2) as work,
            tc.tile_pool(name="small", bufs=4) as small,
        ):
            w_b = singles.tile([P, H], f32)
            nc.sync.dma_start(out=w_b, in_=bcast_rows(w[:, 0]))
            if apply_affine:
                gamma_b = singles.tile([P, H], f32)
                nc.sync.dma_start(out=gamma_b, in_=bcast_rows(gamma[:]))
                beta_b = singles.tile([P, H], f32)
                nc.sync.dma_start(out=beta_b, in_=bcast_rows(beta[:]))
            eps_t = singles.tile([P, 1], f32)
            nc.vector.memset(eps_t, LN_EPS)

            for i in range(NT):
                r0 = i * P
                x0_t = io.tile([P, H], f32, tag="x0")
                x1_t = io.tile([P, H], f32, tag="x1")
                nc.sync.dma_start(out=x0_t, in_=x0[r0 : r0 + P, :])
                nc.sync.dma_start(out=x1_t, in_=x1[r0 : r0 + P, :])

                # s = rowsum(x1 * w); out is a stride-0 dummy (never read)
                dummy = work.tile([P, 1], f32, tag="dummy")
                s = small.tile([P, 1], f32, tag="s")
                nc.vector.scalar_tensor_tensor(
                    out=dummy.broadcast_to([P, H]),
                    in0=x1_t,
                    scalar=1.0,
                    in1=w_b,
                    op0=op.mult,
                    op1=op.mult,
                    accum_out=s,
                )

                # y = (x0 * s) + x0 (same rounding as reference), ysum for mean
                y_t = io.tile([P, H], f32, tag="y")
                ysum = small.tile([P, 1], f32, tag="ysum")
                nc.vector.scalar_tensor_tensor(
                    out=y_t,
                    in0=x0_t,
                    scalar=s,
                    in1=x0_t,
                    op0=op.mult,
                    op1=op.add,
                    accum_out=ysum,
                )

                # negm = -mean(y)
                negm = small.tile([P, 1], f32, tag="negm")
                nc.vector.tensor_scalar_mul(out=negm, in0=ysum, scalar1=-1.0 / H)

                # ss = sum((y - mean)^2); squares go to a junk tile
                junk = work.tile([P, H], f32, tag="junk")
                ss = small.tile([P, 1], f32, tag="ss")
                nc.scalar.activation(
                    out=junk,
                    in_=y_t,
                    func=act_fn.Square,
                    bias=negm,
                    scale=1.0,
                    accum_out=ss,
                )

                # rstd = 1/sqrt(ss/H + eps); b = negm * rstd
                t = small.tile([P, 1], f32, tag="t")
                nc.scalar.activation(
                    out=t, in_=ss, func=act_fn.Sqrt, bias=eps_t, scale=1.0 / H
                )
                r = small.tile([P, 1], f32, tag="r")
                nc.vector.reciprocal(out=r, in_=t)
                b_sc = small.tile([P, 1], f32, tag="b")
                nc.vector.tensor_mul(out=b_sc, in0=negm, in1=r)

                # out = y*rstd + b  (== (y-mean)*rstd)
                out_t = io.tile([P, H], f32, tag="out")
                nc.scalar.activation(
                    out=out_t, in_=y_t, func=act_fn.Identity, bias=b_sc, scale=r
                )

                if apply_affine:
                    nc.vector.scalar_tensor_tensor(
                        out=out_t,
                        in0=out_t,
                        scalar=0.0,
                        in1=gamma_b,
                        op0=op.add,
                        op1=op.mult,
                    )
                    nc.vector.tensor_add(out=out_t, in0=out_t, in1=beta_b)

                nc.sync.dma_start(out=out[r0 : r0 + P, :], in_=out_t)

    nc.compile()
    return nc


LAST_RESULTS = None


def kernel(x0, x1, weight, ln_gamma, ln_beta):
    from concourse.bass_utils import run_bass_kernel_spmd

    global LAST_RESULTS
    x0 = np.asarray(x0, dtype=np.float32)
    x1 = np.asarray(x1, dtype=np.float32)
    weight = np.asarray(weight, dtype=np.float32)
    ln_gamma = np.asarray(ln_gamma, dtype=np.float32)
    ln_beta = np.asarray(ln_beta, dtype=np.float32)

    apply_affine = not (
        np.all(ln_gamma == 1.0) and np.all(ln_beta == 0.0)
    )
    if apply_affine not in _cache:
        _cache[apply_affine] = _build(apply_affine)
    nc = _cache[apply_affine]

    in_maps = []
    for k in range(N_CORES):
        m = {
            "x0": x0[k * ROWS : (k + 1) * ROWS],
            "x1": x1[k * ROWS : (k + 1) * ROWS],
            "weight": weight,
        }
        if apply_affine:
            m["ln_gamma"] = ln_gamma
            m["ln_beta"] = ln_beta
        in_maps.append(m)

    res = run_bass_kernel_spmd(nc, in_maps, core_ids=list(range(N_CORES)))
    LAST_RESULTS = res
    out = np.concatenate([res.results[k]["out"] for k in range(N_CORES)], axis=0)
    return (x0, out)


# revision 5
# speedup vs baseline: 1.1237x; 1.1237x over previous
"""Trainium2 Bass kernel for nn_CrossLayer: out = LayerNorm(x0 * (x1@w) + x0).

Math: s = x1 @ w (per-row scalar), y = x0*s + x0, out = LN(y)*gamma + beta.
Per 128-row tile (H=2048 free dim):
    DVE : scalar_tensor_tensor + accum -> s = rowsum((x1*1)*w_b)  (1 pass, out
          is a stride-0 dummy so the product never lands in SBUF)
    DVE : scalar_tensor_tensor + accum -> y = (x0*s)+x0, ysum     (1 pass;
          bit-identical rounding to the reference's x0*s + x0)
    ACT : activation(Square, bias=-mean, accum) -> ss = sum((y-mean)^2)
    tiny: rstd = 1/sqrt(ss/H + eps);  b = -mean*rstd
    ACT : activation(Identity, scale=rstd, bias=b) -> out = y*rstd + b
Sharding: pure data parallel, rows split across 8 cores; weight/gamma/beta
replicated. gamma==1/beta==0 detected host-side and folded away (the general
affine path applies two extra vector passes).
"""

import numpy as np

B, H = 16384, 2048
N_CORES = 8
ROWS = B // N_CORES          # rows per core
P = 128                      # partitions
NT = ROWS // P               # tiles per core
LN_EPS = 1e-12

_cache = {}


def _build(apply_affine: bool):
    import concourse.bass as bass
    import concourse.bacc as bacc
    import concourse.tile as tile
    from concourse import mybir

    f32 = mybir.dt.float32
    op = mybir.AluOpType
    act_fn = mybir.ActivationFunctionType

    nc = bacc.Bacc("TRN2", target_bir_lowering=False, debug=False)
    x0 = nc.dram_tensor("x0", [ROWS, H], f32, kind="ExternalInput")
    x1 = nc.dram_tensor("x1", [ROWS, H], f32, kind="ExternalInput")
    w = nc.dram_tensor("weight", [H, 1], f32, kind="ExternalInput")
    if apply_affine:
        gamma = nc.dram_tensor("ln_gamma", [H], f32, kind="ExternalInput")
        beta = nc.dram_tensor("ln_beta", [H], f32, kind="ExternalInput")
    out = nc.dram_tensor("out", [ROWS, H], f32, kind="ExternalOutput")

    def bcast_rows(ap_1d):
        # [H] DRAM vector -> [P, H] SBUF tile via partition-stride-0 DMA
        return bass.AP(
            tensor=ap_1d.tensor,
            offset=ap_1d.offset,
            ap=[[0, P]] + list(ap_1d.ap),
        )

    with tile.TileContext(nc) as tc:
        with (
            tc.tile_pool(name="singles", bufs=1) as singles,
            tc.tile_pool(name="io", bufs=3) as io,
            tc.tile_pool(name="work", bufs=2) as work,
            tc.tile_pool(name="small", bufs=4) as small,
        ):
            w_b = singles.tile([P, H], f32)
            nc.sync.dma_start(out=w_b, in_=bcast_rows(w[:, 0]))
            if apply_affine:
                gamma_b = singles.tile([P, H], f32)
                nc.sync.dma_start(out=gamma_b, in_=bcast_rows(gamma[:]))
                beta_b = singles.tile([P, H], f32)
                nc.sync.dma_start(out=beta_b, in_=bcast_rows(beta[:]))
            eps_t = singles.tile([P, 1], f32)
            nc.vector.memset(eps_t, LN_EPS)

            for i in range(NT):
                r0 = i * P
                x0_t = io.tile([P, H], f32, tag="x0")
                x1_t = io.tile([P, H], f32, tag="x1")
                nc.sync.dma_start(out=x0_t, in_=x0[r0 : r0 + P, :])
                nc.sync.dma_start(out=x1_t, in_=x1[r0 : r0 + P, :])

                # s = rowsum(x1 * w); out is a stride-0 dummy (never read)
                dummy = work.tile([P, 1], f32, tag="dummy")
                s = small.tile([P, 1], f32, tag="s")
                nc.vector.scalar_tensor_tensor(
                    out=dummy.broadcast_to([P, H]),
                    in0=x1_t,
                    scalar=1.0,
                    in1=w_b,
                    op0=op.mult,
                    op1=op.mult,
                    accum_out=s,
                )

                # y = (x0 * s) + x0 (same rounding as reference), ysum for mean
                y_t = io.tile([P, H], f32, tag="y")
                ysum = small.tile([P, 1], f32, tag="ysum")
                nc.vector.scalar_tensor_tensor(
                    out=y_t,
                    in0=x0_t,
                    scalar=s,
                    in1=x0_t,
                    op0=op.mult,
                    op1=op.add,
                    accum_out=ysum,
                )

                # negm = -mean(y)
                negm = small.tile([P, 1], f32, tag="negm")
                nc.vector.tensor_scalar_mul(out=negm, in0=ysum, scalar1=-1.0 / H)

                # ss = sum((y - mean)^2); squares go to a junk tile
                junk = work.tile([P, H], f32, tag="junk")
                ss = small.tile([P, 1], f32, tag="ss")
                nc.scalar.activation(
                    out=junk,
                    in_=y_t,
                    func=act_fn.Square,
                    bias=negm,
                    scale=1.0,
                    accum_out=ss,
                )

                # q = ss/H + eps; rstd = 1/sqrt(q) with 2 Newton refinements
                # (ACT Sqrt spline is low-precision; NR restores ~1 ulp)
                q = small.tile([P, 1], f32, tag="q")
                nc.vector.tensor_scalar(
                    out=q, in0=ss, scalar1=1.0 / H, scalar2=LN_EPS,
                    op0=op.mult, op1=op.add,
                )
                t = small.tile([P, 1], f32, tag="t")
                nc.scalar.activation(out=t, in_=q, func=act_fn.Sqrt)
                r = small.tile([P, 1], f32, tag="r")
                nc.vector.reciprocal(out=r, in_=t)
                u = small.tile([P, 1], f32, tag="u")
                for _ in range(2):
                    nc.vector.tensor_mul(out=u, in0=r, in1=r)
                    nc.vector.tensor_mul(out=u, in0=u, in1=q)
                    nc.vector.tensor_scalar(
                        out=u, in0=u, scalar1=-0.5, scalar2=1.5,
                        op0=op.mult, op1=op.add,
                    )
                    nc.vector.tensor_mul(out=r, in0=r, in1=u)
                b_sc = small.tile([P, 1], f32, tag="b")
                nc.vector.tensor_mul(out=b_sc, in0=negm, in1=r)

                # out = y*rstd + b  (== (y-mean)*rstd)
                out_t = io.tile([P, H], f32, tag="out")
                nc.scalar.activation(
                    out=out_t, in_=y_t, func=act_fn.Identity, bias=b_sc, scale=r
                )

                if apply_affine:
                    nc.vector.scalar_tensor_tensor(
                        out=out_t,
                        in0=out_t,
                        scalar=0.0,
                        in1=gamma_b,
                        op0=op.add,
                        op1=op.mult,
                    )
                    nc.vector.tensor_add(out=out_t, in0=out_t, in1=beta_b)

                nc.sync.dma_start(out=out[r0 : r0 + P, :], in_=out_t)

    nc.compile()
    return nc


LAST_RESULTS = None


def kernel(x0, x1, weight, ln_gamma, ln_beta):
    from concourse.bass_utils import run_bass_kernel_spmd

    global LAST_RESULTS
    x0 = np.asarray(x0, dtype=np.float32)
    x1 = np.asarray(x1, dtype=np.float32)
    weight = np.asarray(weight, dtype=np.float32)
    ln_gamma = np.asarray(ln_gamma, dtype=np.float32)
    ln_beta = np.asarray(ln_beta, dtype=np.float32)

    apply_affine = not (
        np.all(ln_gamma == 1.0) and np.all(ln_beta == 0.0)
    )
    if apply_affine not in _cache:
        _cache[apply_affine] = _build(apply_affine)
    nc = _cache[apply_affine]

    in_maps = []
    for k in range(N_CORES):
        m = {
            "x0": x0[k * ROWS : (k + 1) * ROWS],
            "x1": x1[k * ROWS : (k + 1) * ROWS],
            "weight": weight,
        }
        if apply_affine:
            m["ln_gamma"] = ln_gamma
            m["ln_beta"] = ln_beta
        in_maps.append(m)

    res = run_bass_kernel_spmd(nc, in_maps, core_ids=list(range(N_CORES)))
    LAST_RESULTS = res
    out = np.concatenate([res.results[k]["out"] for k in range(N_CORES)], axis=0)
    return (x0, out)
